# revision 1
# baseline (speedup 1.0000x reference)
"""EdgeConv layer program builder for Trainium2 (Bass/Tile).

Math (one EdgeConv layer, PyG semantics, aggr='add' over dst):
  u' = x @ (A_i - A_j).T + ba          (node-level)   A_i|A_j = wa split
  v  = x @ A_j.T                       (node-level)
  t_e = relu(u'[dst_e] + v[src_e])     (edge-level)
  agg[n] = sum_{e: dst_e = n} t_e      (scatter-add)
  conv[n] = agg[n] @ Wb2 + deg[n] * c0 (node-level; BN+linear folded)
  layer1: h = l2norm(relu(conv)); layer2: out = conv

Sharding: edges partitioned by dst across 8 cores (each core owns a
contiguous 128-aligned dst node range) -> outputs are disjoint slices,
no cross-core reduction. v is computed redundantly on every core.

On-chip mapping per 128-edge chunk (dst confined to one 128-node block):
  S^T[m,e] = (dst[e]==m), S[e,m] built by DVE is_equal vs iota
  u-gather: matmul(msg_psum[e,c], lhsT=S^T, rhs=u'_blk)      PE
  v-add:    matmul(msg_psum,      lhsT=I,   rhs=v_gathered)  PE (accum)
  relu:     ACT psum -> t_sb (bf16)
  scatter:  matmul(aggT_psum[c,m], lhsT=t_sb, rhs=S)         PE (accum)
v[src] rows come from an indirect DMA gather (int32 indices) out of an
internal DRAM copy of v written during the node phase.
"""

import sys

sys.path.insert(0, "/opt/trn_rl_repo")

import numpy as np

from concourse import bacc, bass, mybir, tile

F32 = mybir.dt.float32
BF16 = mybir.dt.bfloat16
I32 = mybir.dt.int32
I16 = mybir.dt.int16
BF16_NP = mybir.dt.np(BF16)

C = 128
GRP = 4  # chunks per one-hot build group


def build_layer(n_blocks_total: int, blocks_per_core: int,
                sched_lo: list[int], sched_hi: list[int],
                apply_norm: bool, node_grp: int = 8, gather_blocks: int = 4):
    import os as _os
    ablate = int(_os.environ.get("EDGECONV_ABLATE", "0"))
    """Build one EdgeConv layer program (SPMD, same program all cores)."""
    NBT, BPC = n_blocks_total, blocks_per_core
    blocks, groups, TC = make_layout(sched_lo, sched_hi, BPC, gather_blocks)
    maxw = max((nl + nh for _, nl, _, nh in blocks), default=1)
    nc = bacc.Bacc("TRN2", num_swdge_queues=4)

    # ---- inputs ----
    xt = nc.declare_dram_parameter("xt", [C, NBT * C], BF16, isOutput=False)
    xt_own = nc.declare_dram_parameter("xt_own", [C, BPC * C], BF16, isOutput=False)
    wv_t = nc.declare_dram_parameter("wv_t", [C, C], BF16, isOutput=False)
    wu_t = nc.declare_dram_parameter("wu_t", [C, C], BF16, isOutput=False)
    ba = nc.declare_dram_parameter("ba", [1, C], BF16, isOutput=False)
    wb2 = nc.declare_dram_parameter("wb2", [C, C], BF16, isOutput=False)
    c0 = nc.declare_dram_parameter("c0", [1, C], BF16, isOutput=False)
    iota_col = nc.declare_dram_parameter("iota_col", [C, 1], F32, isOutput=False)
    iota_row4 = nc.declare_dram_parameter("iota_row4", [C, GRP, C], BF16, isOutput=False)
    ident = nc.declare_dram_parameter("ident", [C, C], BF16, isOutput=False)
    deg = nc.declare_dram_parameter("deg", [1, BPC * C], BF16, isOutput=False)
    ones_col = nc.declare_dram_parameter("ones_col", [C, 1], F32, isOutput=False)
    ones_row = nc.declare_dram_parameter("ones_row", [1, C], BF16, isOutput=False)
    ones_row_f32 = nc.declare_dram_parameter("ones_row_f32", [1, C], F32, isOutput=False)
    src16 = nc.declare_dram_parameter("src16", [128, TC * 8], I16, isOutput=False)
    dst_row = nc.declare_dram_parameter("dst_row", [1, TC * C], BF16, isOutput=False)
    dst_col = nc.declare_dram_parameter("dst_col", [C, TC], BF16, isOutput=False)
    out_t = nc.declare_dram_parameter("out_t", [C, BPC * C], F32, isOutput=True)

    max_gchunks = max((nlo + nhi for _, nlo, nhi, _ in groups), default=1)

    u_dram = nc.dram_tensor("u_scratch", [C, BPC * C], BF16)
    v_dram = nc.dram_tensor("v_scratch", [NBT * C, C], BF16)

    # ================= ctx1: node phase =================
    with tile.TileContext(nc) as tc:
        with (
            tc.tile_pool(name="c1const", bufs=1) as c1const,
            tc.tile_pool(name="nodeio", bufs=3) as nodeio,
            tc.tile_pool(name="npsum", bufs=3, space="PSUM") as npsum,
        ):
            wv_sb = c1const.tile([C, C], BF16, tag="wv")
            nc.sync.dma_start(out=wv_sb[:], in_=wv_t[:])

            wu_sb = c1const.tile([C, C], BF16, tag="wu")
            nc.sync.dma_start(out=wu_sb[:], in_=wu_t[:])
            ba_sb = c1const.tile([1, C], BF16, tag="ba")
            nc.sync.dma_start(out=ba_sb[:], in_=ba[:])
            onesr1_sb = c1const.tile([1, C], BF16, tag="onesr1")
            nc.sync.dma_start(out=onesr1_sb[:], in_=ones_row[:])

            for g0 in range(0, NBT, node_grp):
                g1 = min(g0 + node_grp, NBT)
                xt_sb = nodeio.tile([C, node_grp * C], BF16, tag="xt")
                nc.sync.dma_start(out=xt_sb[:, : (g1 - g0) * C],
                                  in_=xt[:, g0 * C: g1 * C])
                for b in range(g0, g1):
                    lhs = xt_sb[:, (b - g0) * C: (b - g0 + 1) * C]
                    vps = npsum.tile([C, C], F32, tag="vps")
                    nc.tensor.matmul(vps[:], lhsT=lhs, rhs=wv_sb[:],
                                     start=True, stop=True)
                    vst = nodeio.tile([C, C], BF16, tag="vst")
                    nc.vector.tensor_copy(out=vst[:], in_=vps[:])
                    nc.sync.dma_start(out=v_dram[b * C: (b + 1) * C, :],
                                      in_=vst[:])

            for g0 in range(0, BPC, node_grp):
                g1 = min(g0 + node_grp, BPC)
                xo_sb = nodeio.tile([C, node_grp * C], BF16, tag="xo")
                nc.sync.dma_start(out=xo_sb[:, : (g1 - g0) * C],
                                  in_=xt_own[:, g0 * C: g1 * C])
                for b in range(g0, g1):
                    lhs = xo_sb[:, (b - g0) * C: (b - g0 + 1) * C]
                    ups = npsum.tile([C, C], F32, tag="vps")
                    nc.tensor.matmul(ups[:], lhsT=lhs, rhs=wu_sb[:],
                                     start=True, stop=False)
                    nc.tensor.matmul(ups[:], lhsT=onesr1_sb[:], rhs=ba_sb[:],
                                     start=False, stop=True)
                    ust = nodeio.tile([C, C], BF16, tag="vst")
                    nc.vector.tensor_copy(out=ust[:], in_=ups[:])
                    nc.sync.dma_start(out=u_dram[:, b * C: (b + 1) * C],
                                      in_=ust[:])

    # ================= ctx2: edge phase =================
    with tile.TileContext(nc) as tc:
        with (
            tc.tile_pool(name="const", bufs=1) as constp,
            tc.tile_pool(name="persist", bufs=1) as persist,
            tc.tile_pool(name="outio", bufs=3) as outio,
            tc.tile_pool(name="edgeio", bufs=3) as edgeio,
            tc.tile_pool(name="dstrp", bufs=2) as dstrp,
            tc.tile_pool(name="msgp", bufs=4, space="PSUM") as msgp,
            tc.tile_pool(name="aggp", bufs=2, space="PSUM") as aggp,
            tc.tile_pool(name="convp", bufs=2, space="PSUM") as convp,
        ):
            wb2_sb = constp.tile([C, C], BF16, tag="wb2")
            nc.sync.dma_start(out=wb2_sb[:], in_=wb2[:])
            c0_sb = constp.tile([1, C], BF16, tag="c0")
            nc.sync.dma_start(out=c0_sb[:], in_=c0[:])
            ic_sb = constp.tile([C, 1], F32, tag="ic")
            nc.sync.dma_start(out=ic_sb[:], in_=iota_col[:])
            ir_sb = constp.tile([C, GRP, C], BF16, tag="ir")
            nc.sync.dma_start(out=ir_sb[:], in_=iota_row4[:])
            id_sb = constp.tile([C, C], BF16, tag="id")
            nc.sync.dma_start(out=id_sb[:], in_=ident[:])
            deg_sb = constp.tile([1, BPC * C], BF16, tag="deg")
            nc.sync.dma_start(out=deg_sb[:], in_=deg[:])
            ones_sb = constp.tile([C, 1], F32, tag="ones")
            nc.sync.dma_start(out=ones_sb[:], in_=ones_col[:])
            onesr_sb = constp.tile([1, C], BF16, tag="onesr")
            nc.sync.dma_start(out=onesr_sb[:], in_=ones_row[:])
            onesrf_sb = constp.tile([1, C], F32, tag="onesrf")
            nc.sync.dma_start(out=onesrf_sb[:], in_=ones_row_f32[:])
            u_sb = persist.tile([C, BPC * C], BF16, tag="u")
            nc.sync.dma_start(out=u_sb[:], in_=u_dram[:])
            srci_sb = persist.tile([128, TC * 8], I16, tag="srci")
            nc.sync.dma_start(out=srci_sb[:], in_=src16[:])
            dstc_sb = persist.tile([C, TC], BF16, tag="dstc")
            nc.sync.dma_start(out=dstc_sb[:], in_=dst_col[:])

            grp_of_block = {}
            for gi_, (g_start, nlo_g, nhi_g, bs) in enumerate(groups):
                grp_of_block[bs[0]] = gi_

            vg_sb = None
            dstrg_sb = None
            vg_base = 0
            gq = [0]
            n_gq = 4
            for b in range(BPC):
                lo0, nl, hi0, nh = blocks[b]
                nch = nl + nh
                if b in grp_of_block:
                    g_start, nlo_g, nhi_g, _ = groups[grp_of_block[b]]
                    vg_base = g_start
                    ng = nlo_g + nhi_g
                    if ng > 0:
                        vg_sb = edgeio.tile([C, max_gchunks, C], BF16, tag="vg")
                        MAXCH = 4   # 512 idx per call (desc-ring-safe)
                        if ablate >= 1:
                            nc.gpsimd.memset(vg_sb[:], 0.0)
                        else:
                         for (cb, cn, base_ap) in (
                                (0, nlo_g, v_dram[:min(SPLIT, NBT * C), :]),
                                (nlo_g, nhi_g,
                                 v_dram[min(SPLIT, NBT * C):, :])):
                             for c0_ in range(0, cn, MAXCH):
                                cw = min(MAXCH, cn - c0_)
                                sl = g_start + cb + c0_
                                nc.gpsimd.dma_gather(
                                    out_ap=vg_sb[:, cb + c0_: cb + c0_ + cw, :],
                                    in_ap=base_ap,
                                    idxs_ap=srci_sb[:, sl * 8: (sl + cw) * 8],
                                    num_idxs=cw * C,
                                    num_idxs_reg=cw * C,
                                    elem_size=C,
                                    queue_num=gq[0] % n_gq)
                                gq[0] += 1
                        dstrg_sb = dstrp.tile([1, max_gchunks * C], BF16,
                                              tag="dstr")
                        nc.sync.dma_start(
                            out=dstrg_sb[:, : ng * C],
                            in_=dst_row[0:1, g_start * C: (g_start + ng) * C])
                aggT = aggp.tile([C, C], F32, tag="agg")
                if nch == 0:
                    agg_sb = outio.tile([C, C], BF16, tag="aggsb")
                    nc.gpsimd.memset(agg_sb[:], 0.0)
                else:
                    sT_sb = edgeio.tile([C, maxw * C], BF16, tag="sT")
                    s_sb = edgeio.tile([C, maxw, C], BF16, tag="s")
                    # builds over the block's lo range then hi range
                    loc = 0
                    for (r0, rn) in ((lo0, nl), (hi0, nh)):
                        for j0 in range(0, rn, GRP):
                            gw = min(GRP, rn - j0)
                            gslot = r0 + j0
                            bc_ps = msgp.tile([C, GRP * C], F32, tag="msg")
                            nc.tensor.matmul(
                                bc_ps[:, : gw * C], lhsT=onesr_sb[:],
                                rhs=dstrg_sb[0:1, (gslot - vg_base) * C:
                                             (gslot - vg_base + gw) * C],
                                start=True, stop=True)
                            nc.vector.tensor_scalar(
                                out=sT_sb[:, (loc + j0) * C:
                                          (loc + j0 + gw) * C],
                                in0=bc_ps[:, : gw * C],
                                scalar1=ic_sb[:],
                                scalar2=None,
                                op0=mybir.AluOpType.is_equal)
                            nc.vector.tensor_tensor(
                                out=s_sb[:, loc + j0: loc + j0 + gw, :],
                                in0=ir_sb[:, :gw, :],
                                in1=dstc_sb[:, gslot: gslot + gw]
                                    .to_broadcast([C, gw, C]),
                                op=mybir.AluOpType.is_equal)
                        loc += rn
                    loc = 0
                    jj_list = (list(range(lo0, lo0 + nl))
                               + list(range(hi0, hi0 + nh)))
                    for j, gslot in enumerate(jj_list):
                        msg = msgp.tile([C, GRP * C], F32, tag="msg")
                        nc.tensor.matmul(msg[:, :C],
                                         lhsT=sT_sb[:, j * C: (j + 1) * C],
                                         rhs=u_sb[:, b * C: (b + 1) * C],
                                         start=True, stop=False)
                        nc.tensor.matmul(msg[:, :C], lhsT=id_sb[:],
                                         rhs=vg_sb[:, gslot - vg_base, :],
                                         start=False, stop=True)
                        t_sb = edgeio.tile([C, C], BF16, tag="t")
                        nc.scalar.activation(out=t_sb[:], in_=msg[:, :C],
                                             func=mybir.ActivationFunctionType.Relu)
                        nc.tensor.matmul(aggT[:], lhsT=t_sb[:],
                                         rhs=s_sb[:, j, :],
                                         start=(j == 0), stop=(j == nch - 1))
                    agg_sb = outio.tile([C, C], BF16, tag="aggsb")
                    nc.vector.tensor_copy(out=agg_sb[:], in_=aggT[:])

                cps = convp.tile([C, C], F32, tag="conv")
                nc.tensor.matmul(cps[:], lhsT=wb2_sb[:], rhs=agg_sb[:],
                                 start=True, stop=False)
                nc.tensor.matmul(cps[:], lhsT=c0_sb[:],
                                 rhs=deg_sb[0:1, b * C: (b + 1) * C],
                                 start=False, stop=True)

                o_sb = outio.tile([C, C], F32, tag="o")
                if apply_norm:
                    h_sb = outio.tile([C, C], F32, tag="h")
                    nc.scalar.activation(out=h_sb[:], in_=cps[:],
                                         func=mybir.ActivationFunctionType.Relu)
                    sq_sb = outio.tile([C, C], F32, tag="sq")
                    nc.vector.tensor_tensor(out=sq_sb[:], in0=h_sb[:],
                                            in1=h_sb[:],
                                            op=mybir.AluOpType.mult)
                    ssq = convp.tile([1, C], F32, tag="conv")
                    nc.tensor.matmul(ssq[:], lhsT=ones_sb[:], rhs=sq_sb[:],
                                     start=True, stop=True)
                    nrm = outio.tile([1, C], F32, tag="nrm")
                    nc.scalar.activation(out=nrm[:], in_=ssq[:],
                                         func=mybir.ActivationFunctionType.Sqrt)
                    nc.vector.tensor_scalar(out=nrm[:], in0=nrm[:],
                                            scalar1=1e-12, scalar2=None,
                                            op0=mybir.AluOpType.max)
                    nc.vector.reciprocal(out=nrm[:], in_=nrm[:])
                    inv_ps = msgp.tile([C, GRP * C], F32, tag="msg")
                    nc.tensor.matmul(inv_ps[:, :C], lhsT=onesrf_sb[:],
                                     rhs=nrm[:], start=True, stop=True)
                    nc.vector.tensor_tensor(out=o_sb[:], in0=h_sb[:],
                                            in1=inv_ps[:, :C],
                                            op=mybir.AluOpType.mult)
                else:
                    nc.scalar.activation(out=o_sb[:], in_=cps[:],
                                         func=mybir.ActivationFunctionType.Copy)
                nc.sync.dma_start(out=out_t[:, b * C: (b + 1) * C], in_=o_sb[:])

    nc.compile()   # bacc passes incl. generate_event_semaphores (1-wait limit)
    return nc


def _split_excess_dma_waits(nc, max_waits: int = 1):
    """Walrus DMA codegen rejects multiple sync waits on one DMA instruction.
    Move the excess onto a NoOp on the same engine right before it."""
    k = 0
    for blk in nc.m.functions[0].blocks:
        while True:
            insts = blk.instructions
            fixed = False
            for i, inst in enumerate(insts):
                si = inst.sync_info
                if (si is not None and len(si.on_wait) > max_waits
                        and isinstance(inst, mybir.InstDMACopy)):
                    w = list(si.on_wait)
                    noop = mybir.InstNoOp(
                        name=f"I-waitfix-{k}", engine=inst.engine,
                        sync_info=mybir.SyncInfo(on_wait=w[:-max_waits],
                                                 on_update=[]))
                    k += 1
                    inst.sync_info = mybir.SyncInfo(
                        on_wait=w[-max_waits:], on_update=list(si.on_update))
                    blk.instructions.insert(i, noop)
                    fixed = True
                    break
            if not fixed:
                break


def sched_max_w(sched):
    m = max(sched) if sched else 1
    return max(m, 1)


# ---------------- host-side data prep ----------------

SPLIT = 32768


def make_layout(sched_lo, sched_hi, bpc, gather_blocks=4):
    """Group-major slot order: per gather group, all lo slots (block-major)
    then all hi slots. Returns per-block (lo_start, nlo, hi_start, nhi),
    group list (chunk_start, nlo_g, nhi_g, blocks)."""
    blocks = []
    groups = []
    pos = 0
    b = 0
    while b < bpc:
        bs = list(range(b, min(b + gather_blocks, bpc)))
        g_start = pos
        lo_starts = {}
        for bb in bs:
            lo_starts[bb] = pos
            pos += sched_lo[bb]
        nlo_g = pos - g_start
        hi_starts = {}
        for bb in bs:
            hi_starts[bb] = pos
            pos += sched_hi[bb]
        nhi_g = pos - g_start - nlo_g
        for bb in bs:
            blocks.append((lo_starts[bb], sched_lo[bb],
                           hi_starts[bb], sched_hi[bb]))
        groups.append((g_start, nlo_g, nhi_g, bs))
        b += gather_blocks
    return blocks, groups, pos


def prep_edges(src, dst, n_cores, bpc, gather_blocks=4):
    """Partition edges by dst core/block, split each block's edges into
    lo (src < SPLIT) and hi chunks for int16 dma_gather indexing."""
    npc = bpc * C
    order = np.argsort(dst, kind="stable")
    src_s, dst_s = src[order], dst[order]
    core_lists = []
    nlo = np.zeros((n_cores, bpc), np.int64)
    nhi = np.zeros((n_cores, bpc), np.int64)
    for k in range(n_cores):
        lo_ = np.searchsorted(dst_s, k * npc, side="left")
        hi_ = np.searchsorted(dst_s, (k + 1) * npc, side="left")
        s_k, d_k = src_s[lo_:hi_], dst_s[lo_:hi_] - k * npc
        blk = d_k // C
        per_blk = []
        for b in range(bpc):
            m = blk == b
            sb, db = s_k[m], d_k[m] - b * C
            isl = sb < SPLIT
            per_blk.append(((sb[isl], db[isl]), (sb[~isl], db[~isl])))
            nlo[k, b] = isl.sum()
            nhi[k, b] = (~isl).sum()
        core_lists.append(per_blk)
    sched_lo = [int(x) for x in np.ceil(nlo.max(axis=0) / C).astype(np.int64)]
    sched_hi = [int(x) for x in np.ceil(nhi.max(axis=0) / C).astype(np.int64)]
    blocks, groups, TC = make_layout(sched_lo, sched_hi, bpc, gather_blocks)

    per_core = []
    for k in range(n_cores):
        si16 = np.zeros((16, TC * 8), np.int16)
        db_ = np.full((TC, C), 200.0, np.float64)
        for b in range(bpc):
            (slo, sdlo), (shi, sdhi) = core_lists[k][b]
            lo0, nl, hi0, nh = blocks[b]
            for (vals, dvals, base, nslots, off) in (
                    (slo, sdlo, lo0, nl, 0), (shi, sdhi, hi0, nh, SPLIT)):
                n = len(vals)
                if nslots == 0:
                    continue
                idx = np.arange(n)
                ch = base + idx // C
                lane = idx % C
                iv = (vals - off).astype(np.int16)
                si16[lane % 16, ch * 8 + lane // 16] = iv
                db_[ch, lane] = dvals
        full = np.zeros((128, TC * 8), np.int16)
        for rr in range(8):
            full[rr * 16: (rr + 1) * 16] = si16
        per_core.append({
            "src16": full,                                       # [128, TC*8]
            "dst_col": np.ascontiguousarray(db_.T.astype(BF16_NP)),
            "dst_row": np.ascontiguousarray(
                db_.reshape(1, -1).astype(BF16_NP)),
        })
    return sched_lo, sched_hi, per_core


def fold_weights(wa, ba_, g, be, rm, rv, wb, bb, bn_eps=1e-5):
    wa = wa.astype(np.float64)
    A_i, A_j = wa[:, :C], wa[:, C:]
    s = g.astype(np.float64) / np.sqrt(rv.astype(np.float64) + bn_eps)
    wb64 = wb.astype(np.float64)
    wu_t = (A_i - A_j).T
    wv_t = A_j.T
    wb2 = s[:, None] * wb64.T
    c0 = bb.astype(np.float64) + (be.astype(np.float64) - rm.astype(np.float64) * s) @ wb64.T
    return (wu_t.astype(BF16_NP), wv_t.astype(BF16_NP),
            ba_.astype(BF16_NP).reshape(1, C),
            wb2.astype(BF16_NP), c0.astype(BF16_NP).reshape(1, C))


def make_consts():
    ic = np.arange(C, dtype=np.float32).reshape(C, 1)
    ir4 = np.tile(np.arange(C, dtype=np.float64), (C, GRP, 1)).astype(BF16_NP)
    ident = np.eye(C, dtype=np.float64).astype(BF16_NP)
    ones = np.ones((C, 1), dtype=np.float32)
    return ic, ir4, ident, ones


# ======================================================================
# Full-problem kernel: 2-layer EdgeConv encoder, N=50000, E=600000, C=128
# ======================================================================

import os

N_NODES = 50000
N_EDGES = 600000
CORES = 8
BPC = 49                  # blocks per core
NBT = CORES * BPC         # 392 blocks total
NP = NBT * C              # padded node count 50176
BN_EPS = 1e-5

LAST = {}                 # timing/info stash for test harness


def _prep_all(x, edge_index):
    src = np.asarray(edge_index[0], np.int64).astype(np.int32)
    dst = np.asarray(edge_index[1], np.int64).astype(np.int32)
    sched_lo, sched_hi, per_core = prep_edges(src, dst, CORES, BPC)
    deg_full = np.bincount(dst, minlength=NP).astype(np.float64)
    x_pad = np.zeros((NP, C), np.float32)
    x_pad[:N_NODES] = x
    xt = np.ascontiguousarray(x_pad.T).astype(BF16_NP)
    return sched_lo, sched_hi, per_core, deg_full, xt


def _layer_inputs(xt_bf16, per_core, deg_full, wset):
    wu_t, wv_t, ba_f, wb2, c0 = wset
    ic, ir4, ident, ones = make_consts()
    onesr = np.ones((1, C), dtype=BF16_NP)
    onesrf = np.ones((1, C), np.float32)
    in_maps = []
    for k in range(CORES):
        npc = BPC * C
        in_maps.append({
            "xt": xt_bf16,
            "xt_own": np.ascontiguousarray(xt_bf16[:, k * npc: (k + 1) * npc]),
            "wv_t": wv_t, "wu_t": wu_t, "ba": ba_f, "wb2": wb2, "c0": c0,
            "iota_col": ic, "iota_row4": ir4, "ident": ident,
            "deg": np.ascontiguousarray(
                deg_full[k * npc: (k + 1) * npc].reshape(1, npc).astype(BF16_NP)),
            "ones_col": ones, "ones_row": onesr, "ones_row_f32": onesrf,
            "src16": per_core[k]["src16"],
            "dst_row": per_core[k]["dst_row"],
            "dst_col": per_core[k]["dst_col"],
        })
    return in_maps


_NTFF_HOOK = None


def _get_ntff_hook():
    """Recreate the axon NTFF profile hook (antenv.axon_hooks is absent
    in this image; trn_boot has the ctypes implementation)."""
    global _NTFF_HOOK
    if _NTFF_HOOK is None:
        sys.path.insert(0, "/root/.axon_site")
        from trn_agent_boot.trn_boot import _ntff_profile_via_ctypes
        _NTFF_HOOK = _ntff_profile_via_ctypes("/opt/axon/libaxon_pjrt.so")
    return _NTFF_HOOK


def _run(nc, in_maps):
    import tempfile
    from concourse import bass2jax
    trace = bool(int(os.environ.get("EDGECONV_TRACE", "0")))
    hook = _get_ntff_hook() if trace else None
    if hook is None:
        results = bass2jax.run_bass_via_pjrt(nc, in_maps, n_cores=CORES)
        LAST.setdefault("exec_ns", []).append(None)
        return results
    neff_dir = tempfile.mkdtemp(prefix="edgeconv_ntff_")
    with hook(neff_dir, [0]):
        results = bass2jax.run_bass_via_pjrt(nc, in_maps, n_cores=CORES)
    exec_ns = None
    try:
        import glob as _glob
        import gauge.profiler
        from concourse._compat import FishPath
        if _glob.glob(os.path.join(neff_dir, "*_body*.ntff")):
            profile = gauge.profiler.Profile(
                profile_path=FishPath(neff_dir), kernel_dev_mode=True,
                profile_on_exit=False, bass_kernel=nc.m,
                offline_processing=True, fname="*_body*")
            pr = profile.to_perfetto(model_index=(0,))
            if pr:
                exec_ns = pr[0].exec_time_ns
                LAST.setdefault("trace_paths", []).append(pr[0].trace_path)
    except Exception as e:  # profiling must never break the kernel
        LAST.setdefault("trace_errors", []).append(repr(e))
    LAST.setdefault("neff_dirs", []).append(neff_dir)
    LAST.setdefault("exec_ns", []).append(exec_ns)
    return results


def kernel(**inputs):
    x = np.asarray(inputs["x"], np.float32)
    edge_index = np.asarray(inputs["edge_index"])
    sched_lo, sched_hi, per_core, deg_full, xt = _prep_all(x, edge_index)

    w1 = fold_weights(np.asarray(inputs["w1a"]), np.asarray(inputs["b1a"]),
                      np.asarray(inputs["g1"]), np.asarray(inputs["be1"]),
                      np.asarray(inputs["rm1"]), np.asarray(inputs["rv1"]),
                      np.asarray(inputs["w1b"]), np.asarray(inputs["b1b"]),
                      BN_EPS)
    w2 = fold_weights(np.asarray(inputs["w2a"]), np.asarray(inputs["b2a"]),
                      np.asarray(inputs["g2"]), np.asarray(inputs["be2"]),
                      np.asarray(inputs["rm2"]), np.asarray(inputs["rv2"]),
                      np.asarray(inputs["w2b"]), np.asarray(inputs["b2b"]),
                      BN_EPS)

    nc1 = build_layer(NBT, BPC, sched_lo, sched_hi, apply_norm=True)
    r1 = _run(nc1, _layer_inputs(xt, per_core, deg_full, w1))
    hT = np.concatenate([np.asarray(r["out_t"], np.float32) for r in r1], axis=1)

    nc2 = build_layer(NBT, BPC, sched_lo, sched_hi, apply_norm=False)
    r2 = _run(nc2, _layer_inputs(hT.astype(BF16_NP), per_core, deg_full, w2))
    outT = np.concatenate([np.asarray(r["out_t"], np.float32) for r in r2], axis=1)

    return np.ascontiguousarray(outT.T[:N_NODES]).astype(np.float32)



# revision 4
# speedup vs baseline: 1.8330x; 1.8330x over previous
"""EdgeConv layer program builder for Trainium2 (Bass/Tile).

Math (one EdgeConv layer, PyG semantics, aggr='add' over dst):
  u' = x @ (A_i - A_j).T + ba          (node-level)   A_i|A_j = wa split
  v  = x @ A_j.T                       (node-level)
  t_e = relu(u'[dst_e] + v[src_e])     (edge-level)
  agg[n] = sum_{e: dst_e = n} t_e      (scatter-add)
  conv[n] = agg[n] @ Wb2 + deg[n] * c0 (node-level; BN+linear folded)
  layer1: h = l2norm(relu(conv)); layer2: out = conv

Sharding: edges partitioned by dst across 8 cores (each core owns a
contiguous 128-aligned dst node range) -> outputs are disjoint slices,
no cross-core reduction. v is computed redundantly on every core.

On-chip mapping per 128-edge chunk (dst confined to one 128-node block):
  S^T[m,e] = (dst[e]==m), S[e,m] built by DVE is_equal vs iota
  u-gather: matmul(msg_psum[e,c], lhsT=S^T, rhs=u'_blk)      PE
  v-add:    DVE tensor_tensor add (msg_psum + v_gathered)    DVE
  relu:     ACT -> t_sb (bf16), grouped over 8 chunks
  scatter:  matmul(aggT_psum[c,m], lhsT=t_sb, rhs=S)         PE (accum)
v[src] rows come from an indirect DMA gather (int16 indices) out of an
internal DRAM copy of v written during the node phase. v is stored
column-block-major ([C, NBT*C], i.e. row r = (n%128)*NBT + n//128 of a
flat [128*NBT, C] view) so node-phase writes are 2KB-contiguous; the
int16 lo/hi index split is on (n%128) < 83.
"""

import sys

sys.path.insert(0, "/opt/trn_rl_repo")

import numpy as np

from concourse import bacc, bass, mybir, tile

F32 = mybir.dt.float32
BF16 = mybir.dt.bfloat16
I32 = mybir.dt.int32
I16 = mybir.dt.int16
BF16_NP = mybir.dt.np(BF16)

C = 128
GRP = 8       # chunks per one-hot build / relu group
TAIL = 4      # blocks per conv/norm/output tail group
NODE_GRP = 8  # blocks per node-phase DMA
P_SPLIT = 83  # lo rows: (n%128) < 83  ->  idx = (n%128)*NBT + n//128 <= 32535


def build_layer(n_blocks_total: int, blocks_per_core: int,
                sched_lo: list[int], sched_hi: list[int],
                apply_norm: bool, gather_blocks: int = 4):
    """Build one EdgeConv layer program (SPMD, same program all cores)."""
    NBT, BPC = n_blocks_total, blocks_per_core
    blocks, groups, TC = make_layout(sched_lo, sched_hi, BPC, gather_blocks)
    nc = bacc.Bacc("TRN2", num_swdge_queues=4)
    OUT_DT = BF16 if apply_norm else F32

    # ---- inputs ----
    xt = nc.declare_dram_parameter("xt", [C, NBT * C], BF16, isOutput=False)
    xt_own = nc.declare_dram_parameter("xt_own", [C, BPC * C], BF16, isOutput=False)
    wv_t = nc.declare_dram_parameter("wv_t", [C, C], BF16, isOutput=False)
    wu_t = nc.declare_dram_parameter("wu_t", [C, C], BF16, isOutput=False)
    ba = nc.declare_dram_parameter("ba", [1, C], BF16, isOutput=False)
    wb2 = nc.declare_dram_parameter("wb2", [C, C], BF16, isOutput=False)
    c0 = nc.declare_dram_parameter("c0", [1, C], BF16, isOutput=False)
    iota_col = nc.declare_dram_parameter("iota_col", [C, 1], F32, isOutput=False)
    iota_row = nc.declare_dram_parameter("iota_row", [C, GRP, C], BF16, isOutput=False)
    deg = nc.declare_dram_parameter("deg", [1, BPC * C], BF16, isOutput=False)
    ones_col = nc.declare_dram_parameter("ones_col", [C, 1], F32, isOutput=False)
    ones_row = nc.declare_dram_parameter("ones_row", [1, C], BF16, isOutput=False)
    ones_row_f32 = nc.declare_dram_parameter("ones_row_f32", [1, C], F32, isOutput=False)
    src16 = nc.declare_dram_parameter("src16", [128, TC * 8], I16, isOutput=False)
    dst_row = nc.declare_dram_parameter("dst_row", [1, TC * C], BF16, isOutput=False)
    dst_col = nc.declare_dram_parameter("dst_col", [C, TC], BF16, isOutput=False)
    out_t = nc.declare_dram_parameter("out_t", [C, BPC * C], OUT_DT, isOutput=True)

    max_gchunks = max((nlo + nhi for _, nlo, nhi, _ in groups), default=1)

    # v scratch, column-block layout: v_dram[p, b*C + c] = v[b*128+p, c]
    u_dram = nc.dram_tensor("u_scratch", [C, BPC * C], BF16)
    v_dram = nc.dram_tensor("v_scratch", [C, NBT * C], BF16)

    # ================= ctx1: node phase =================
    with tile.TileContext(nc) as tc:
        with (
            tc.tile_pool(name="c1const", bufs=1) as c1const,
            tc.tile_pool(name="nodeio", bufs=3) as nodeio,
            tc.tile_pool(name="npsum", bufs=4, space="PSUM") as npsum,
        ):
            wv_sb = c1const.tile([C, C], BF16, tag="wv")
            nc.sync.dma_start(out=wv_sb[:], in_=wv_t[:])
            wu_sb = c1const.tile([C, C], BF16, tag="wu")
            nc.sync.dma_start(out=wu_sb[:], in_=wu_t[:])
            ba_sb = c1const.tile([1, C], BF16, tag="ba")
            nc.sync.dma_start(out=ba_sb[:], in_=ba[:])
            onesr1_sb = c1const.tile([1, C], BF16, tag="onesr1")
            nc.sync.dma_start(out=onesr1_sb[:], in_=ones_row[:])

            kcopy = 0
            for g0 in range(0, NBT, NODE_GRP):
                g1 = min(g0 + NODE_GRP, NBT)
                gw = g1 - g0
                xt_sb = nodeio.tile([C, NODE_GRP, C], BF16, tag="xt")
                nc.sync.dma_start(out=xt_sb[:, :gw, :],
                                  in_=xt[:, g0 * C: g1 * C])
                vst8 = nodeio.tile([C, NODE_GRP, C], BF16, tag="vst")
                for b in range(g0, g1):
                    vps = npsum.tile([C, C], F32, tag="vps")
                    nc.tensor.matmul(vps[:], lhsT=xt_sb[:, b - g0, :],
                                     rhs=wv_sb[:], start=True, stop=True)
                    if kcopy % 2 == 0:
                        nc.vector.tensor_copy(out=vst8[:, b - g0, :], in_=vps[:])
                    else:
                        nc.scalar.activation(
                            out=vst8[:, b - g0, :], in_=vps[:],
                            func=mybir.ActivationFunctionType.Copy)
                    kcopy += 1
                nc.sync.dma_start(out=v_dram[:, g0 * C: g1 * C],
                                  in_=vst8[:, :gw, :])

            for g0 in range(0, BPC, NODE_GRP):
                g1 = min(g0 + NODE_GRP, BPC)
                gw = g1 - g0
                xo_sb = nodeio.tile([C, NODE_GRP, C], BF16, tag="xo")
                nc.sync.dma_start(out=xo_sb[:, :gw, :],
                                  in_=xt_own[:, g0 * C: g1 * C])
                ust8 = nodeio.tile([C, NODE_GRP, C], BF16, tag="ust")
                for b in range(g0, g1):
                    ups = npsum.tile([C, C], F32, tag="vps")
                    nc.tensor.matmul(ups[:], lhsT=xo_sb[:, b - g0, :],
                                     rhs=wu_sb[:], start=True, stop=False)
                    nc.tensor.matmul(ups[:], lhsT=onesr1_sb[:], rhs=ba_sb[:],
                                     start=False, stop=True)
                    if kcopy % 2 == 0:
                        nc.vector.tensor_copy(out=ust8[:, b - g0, :], in_=ups[:])
                    else:
                        nc.scalar.activation(
                            out=ust8[:, b - g0, :], in_=ups[:],
                            func=mybir.ActivationFunctionType.Copy)
                    kcopy += 1
                nc.sync.dma_start(out=u_dram[:, g0 * C: g1 * C],
                                  in_=ust8[:, :gw, :])

    # flat row views of v for the gather: row r = p*NBT + b, 256B each
    v_rows = v_dram.rearrange("p (b c) -> (p b) c", c=C)
    v_rows_lo = v_rows[: P_SPLIT * NBT, :]
    v_rows_hi = v_rows[P_SPLIT * NBT:, :]

    # ================= ctx2: edge phase =================
    with tile.TileContext(nc) as tc:
        with (
            tc.tile_pool(name="const", bufs=1) as constp,
            tc.tile_pool(name="persist", bufs=1) as persist,
            tc.tile_pool(name="outio", bufs=3) as outio,
            tc.tile_pool(name="edgeio", bufs=3) as edgeio,
            tc.tile_pool(name="dstrp", bufs=2) as dstrp,
            tc.tile_pool(name="msgp", bufs=2, space="PSUM") as msgp,
            tc.tile_pool(name="aggp", bufs=2, space="PSUM") as aggp,
            tc.tile_pool(name="convp", bufs=1, space="PSUM") as convp,
        ):
            wb2_sb = constp.tile([C, C], BF16, tag="wb2")
            nc.sync.dma_start(out=wb2_sb[:], in_=wb2[:])
            c0_sb = constp.tile([1, C], BF16, tag="c0")
            nc.sync.dma_start(out=c0_sb[:], in_=c0[:])
            ic_sb = constp.tile([C, 1], F32, tag="ic")
            nc.sync.dma_start(out=ic_sb[:], in_=iota_col[:])
            ir_sb = constp.tile([C, GRP, C], BF16, tag="ir")
            nc.sync.dma_start(out=ir_sb[:], in_=iota_row[:])
            deg_sb = constp.tile([1, BPC * C], BF16, tag="deg")
            nc.sync.dma_start(out=deg_sb[:], in_=deg[:])
            ones_sb = constp.tile([C, 1], F32, tag="ones")
            nc.sync.dma_start(out=ones_sb[:], in_=ones_col[:])
            onesr_sb = constp.tile([1, C], BF16, tag="onesr")
            nc.sync.dma_start(out=onesr_sb[:], in_=ones_row[:])
            onesrf_sb = constp.tile([1, C], F32, tag="onesrf")
            nc.sync.dma_start(out=onesrf_sb[:], in_=ones_row_f32[:])
            u_sb = persist.tile([C, BPC * C], BF16, tag="u")
            nc.sync.dma_start(out=u_sb[:], in_=u_dram[:])
            srci_sb = persist.tile([128, TC * 8], I16, tag="srci")
            nc.sync.dma_start(out=srci_sb[:], in_=src16[:])
            dstc_sb = persist.tile([C, TC], BF16, tag="dstc")
            nc.sync.dma_start(out=dstc_sb[:], in_=dst_col[:])

            grp_of_block = {}
            for gi_, (g_start, nlo_g, nhi_g, bs) in enumerate(groups):
                grp_of_block[bs[0]] = gi_

            vg_sb = None
            dstrg_sb = None
            vg_base = 0
            gq = [0]
            n_gq = 4
            agg4_sb = None
            for b in range(BPC):
                lo0, nl, hi0, nh = blocks[b]
                nch = nl + nh
                if b in grp_of_block:
                    g_start, nlo_g, nhi_g, _ = groups[grp_of_block[b]]
                    vg_base = g_start
                    ng = nlo_g + nhi_g
                    if ng > 0:
                        vg_sb = edgeio.tile([C, max_gchunks, C], BF16, tag="vg")
                        MAXCH = 4   # 512 idx per call (desc-ring-safe)
                        for (cb, cn, base_ap) in (
                                (0, nlo_g, v_rows_lo),
                                (nlo_g, nhi_g, v_rows_hi)):
                            for c0_ in range(0, cn, MAXCH):
                                cw = min(MAXCH, cn - c0_)
                                sl = g_start + cb + c0_
                                nc.gpsimd.dma_gather(
                                    out_ap=vg_sb[:, cb + c0_: cb + c0_ + cw, :],
                                    in_ap=base_ap,
                                    idxs_ap=srci_sb[:, sl * 8: (sl + cw) * 8],
                                    num_idxs=cw * C,
                                    num_idxs_reg=cw * C,
                                    elem_size=C,
                                    queue_num=gq[0] % n_gq)
                                gq[0] += 1
                        dstrg_sb = dstrp.tile([1, max_gchunks * C], BF16,
                                              tag="dstr")
                        nc.sync.dma_start(
                            out=dstrg_sb[:, : ng * C],
                            in_=dst_row[0:1, g_start * C: (g_start + ng) * C])

                if b % TAIL == 0:
                    agg4_sb = outio.tile([C, TAIL, C], BF16, tag="agg4")
                bslot = b % TAIL

                if nch == 0:
                    nc.vector.memset(agg4_sb[:, bslot, :], 0.0)
                else:
                    aggT = aggp.tile([C, C], F32, tag="agg")
                    done = 0
                    for (r0, rn) in ((lo0, nl), (hi0, nh)):
                        for j0 in range(0, rn, GRP):
                            gw = min(GRP, rn - j0)
                            gs = r0 + j0          # first slot of this group
                            # broadcast dst row over partitions (PE);
                            # matmul out <= 512 f32 (one PSUM bank) per call
                            bc_ps = msgp.tile([C, GRP, C], F32, tag="msg")
                            for h0 in range(0, gw, 4):
                                hw = min(4, gw - h0)
                                nc.tensor.matmul(
                                    bc_ps[:, h0: h0 + hw, :]
                                    .rearrange("p a b -> p (a b)"),
                                    lhsT=onesr_sb[:],
                                    rhs=dstrg_sb[0:1,
                                                 (gs - vg_base + h0) * C:
                                                 (gs - vg_base + h0 + hw) * C],
                                    start=True, stop=True)
                            sT = edgeio.tile([C, GRP * C], BF16, tag="sT")
                            nc.vector.tensor_scalar(
                                out=sT[:, : gw * C],
                                in0=bc_ps[:, : gw, :].rearrange("p a b -> p (a b)"),
                                scalar1=ic_sb[:],
                                scalar2=None,
                                op0=mybir.AluOpType.is_equal)
                            s8 = edgeio.tile([C, GRP, C], BF16, tag="s")
                            nc.vector.tensor_tensor(
                                out=s8[:, :gw, :],
                                in0=ir_sb[:, :gw, :],
                                in1=dstc_sb[:, gs: gs + gw]
                                    .to_broadcast([C, gw, C]),
                                op=mybir.AluOpType.is_equal)
                            msg_ps = msgp.tile([C, GRP, C], F32, tag="msg")
                            for j in range(gw):
                                nc.tensor.matmul(
                                    msg_ps[:, j, :],
                                    lhsT=sT[:, j * C: (j + 1) * C],
                                    rhs=u_sb[:, b * C: (b + 1) * C],
                                    start=True, stop=True)
                            sum8 = edgeio.tile([C, GRP, C], BF16, tag="sum")
                            nc.vector.tensor_tensor(
                                out=sum8[:, :gw, :],
                                in0=msg_ps[:, :gw, :],
                                in1=vg_sb[:, gs - vg_base: gs - vg_base + gw, :],
                                op=mybir.AluOpType.add)
                            t8 = edgeio.tile([C, GRP, C], BF16, tag="t")
                            nc.scalar.activation(
                                out=t8[:, :gw, :], in_=sum8[:, :gw, :],
                                func=mybir.ActivationFunctionType.Relu)
                            for j in range(gw):
                                nc.tensor.matmul(
                                    aggT[:], lhsT=t8[:, j, :], rhs=s8[:, j, :],
                                    start=(done + j == 0),
                                    stop=(done + j == nch - 1))
                            done += gw
                    if b % 2 == 0:
                        nc.vector.tensor_copy(out=agg4_sb[:, bslot, :],
                                              in_=aggT[:])
                    else:
                        nc.scalar.activation(
                            out=agg4_sb[:, bslot, :], in_=aggT[:],
                            func=mybir.ActivationFunctionType.Copy)

                # ---- tail: conv + (norm) + output, once per TAIL blocks ----
                if b % TAIL == TAIL - 1 or b == BPC - 1:
                    bg0 = (b // TAIL) * TAIL
                    w = b - bg0 + 1
                    conv4 = convp.tile([C, TAIL, C], F32, tag="conv")
                    cflat = conv4[:, :w, :].rearrange("p a b -> p (a b)")
                    nc.tensor.matmul(cflat, lhsT=c0_sb[:],
                                     rhs=deg_sb[0:1, bg0 * C: (bg0 + w) * C],
                                     start=True, stop=False)
                    nc.tensor.matmul(
                        cflat, lhsT=wb2_sb[:],
                        rhs=agg4_sb[:, :w, :].rearrange("p a b -> p (a b)"),
                        start=False, stop=True)
                    o4 = outio.tile([C, TAIL, C], OUT_DT, tag="o4")
                    if apply_norm:
                        h4 = outio.tile([C, TAIL, C], F32, tag="h4")
                        nc.scalar.activation(
                            out=h4[:, :w, :], in_=conv4[:, :w, :],
                            func=mybir.ActivationFunctionType.Relu)
                        sq4 = outio.tile([C, TAIL, C], F32, tag="sq4")
                        nc.vector.tensor_tensor(
                            out=sq4[:, :w, :], in0=h4[:, :w, :],
                            in1=h4[:, :w, :], op=mybir.AluOpType.mult)
                        ssq = convp.tile([1, TAIL * C], F32, tag="ssq")
                        nc.tensor.matmul(
                            ssq[:, : w * C], lhsT=ones_sb[:],
                            rhs=sq4[:, :w, :].rearrange("p a b -> p (a b)"),
                            start=True, stop=True)
                        nrm = outio.tile([1, TAIL * C], F32, tag="nrm")
                        nc.scalar.activation(
                            out=nrm[:, : w * C], in_=ssq[:, : w * C],
                            func=mybir.ActivationFunctionType.Sqrt)
                        nc.vector.tensor_scalar(
                            out=nrm[:, : w * C], in0=nrm[:, : w * C],
                            scalar1=1e-12, scalar2=None,
                            op0=mybir.AluOpType.max)
                        nc.vector.reciprocal(out=nrm[:, : w * C],
                                             in_=nrm[:, : w * C])
                        inv4 = convp.tile([C, TAIL, C], F32, tag="conv")
                        nc.tensor.matmul(
                            inv4[:, :w, :].rearrange("p a b -> p (a b)"),
                            lhsT=onesrf_sb[:], rhs=nrm[:, : w * C],
                            start=True, stop=True)
                        nc.vector.tensor_tensor(
                            out=o4[:, :w, :], in0=h4[:, :w, :],
                            in1=inv4[:, :w, :],
                            op=mybir.AluOpType.mult)
                    else:
                        nc.scalar.activation(
                            out=o4[:, :w, :], in_=conv4[:, :w, :],
                            func=mybir.ActivationFunctionType.Copy)
                    nc.sync.dma_start(out=out_t[:, bg0 * C: (bg0 + w) * C],
                                      in_=o4[:, :w, :])

    nc.compile()   # bacc passes incl. generate_event_semaphores (1-wait limit)
    return nc


# ---------------- host-side data prep ----------------


def make_layout(sched_lo, sched_hi, bpc, gather_blocks=4):
    """Group-major slot order: per gather group, all lo slots (block-major)
    then all hi slots. Returns per-block (lo_start, nlo, hi_start, nhi),
    group list (chunk_start, nlo_g, nhi_g, blocks)."""
    blocks = []
    groups = []
    pos = 0
    b = 0
    while b < bpc:
        bs = list(range(b, min(b + gather_blocks, bpc)))
        g_start = pos
        lo_starts = {}
        for bb in bs:
            lo_starts[bb] = pos
            pos += sched_lo[bb]
        nlo_g = pos - g_start
        hi_starts = {}
        for bb in bs:
            hi_starts[bb] = pos
            pos += sched_hi[bb]
        nhi_g = pos - g_start - nlo_g
        for bb in bs:
            blocks.append((lo_starts[bb], sched_lo[bb],
                           hi_starts[bb], sched_hi[bb]))
        groups.append((g_start, nlo_g, nhi_g, bs))
        b += gather_blocks
    return blocks, groups, pos


def prep_edges(src, dst, n_cores, bpc, nbt, gather_blocks=4):
    """Partition edges by dst core/block, split each block's edges into
    lo ((src%128) < P_SPLIT) and hi chunks for int16 dma_gather indexing.
    Gather index of node n: (n%128)*nbt + n//128 (column-block v layout)."""
    npc = bpc * C
    order = np.argsort(dst, kind="stable")
    src_s, dst_s = src[order], dst[order]
    core_lists = []
    nlo = np.zeros((n_cores, bpc), np.int64)
    nhi = np.zeros((n_cores, bpc), np.int64)
    for k in range(n_cores):
        lo_ = np.searchsorted(dst_s, k * npc, side="left")
        hi_ = np.searchsorted(dst_s, (k + 1) * npc, side="left")
        s_k, d_k = src_s[lo_:hi_], dst_s[lo_:hi_] - k * npc
        blk = d_k // C
        per_blk = []
        for b in range(bpc):
            m = blk == b
            sb, db = s_k[m], d_k[m] - b * C
            sidx = (sb % 128) * nbt + sb // 128
            isl = (sb % 128) < P_SPLIT
            per_blk.append(((sidx[isl], db[isl]), (sidx[~isl], db[~isl])))
            nlo[k, b] = isl.sum()
            nhi[k, b] = (~isl).sum()
        core_lists.append(per_blk)
    sched_lo = [int(x) for x in np.ceil(nlo.max(axis=0) / C).astype(np.int64)]
    sched_hi = [int(x) for x in np.ceil(nhi.max(axis=0) / C).astype(np.int64)]
    blocks, groups, TC = make_layout(sched_lo, sched_hi, bpc, gather_blocks)

    hi_base = P_SPLIT * nbt
    per_core = []
    for k in range(n_cores):
        si16 = np.zeros((16, TC * 8), np.int16)
        db_ = np.full((TC, C), 200.0, np.float64)
        for b in range(bpc):
            (slo, sdlo), (shi, sdhi) = core_lists[k][b]
            lo0, nl, hi0, nh = blocks[b]
            for (vals, dvals, base, nslots, off) in (
                    (slo, sdlo, lo0, nl, 0), (shi, sdhi, hi0, nh, hi_base)):
                n = len(vals)
                if nslots == 0:
                    continue
                idx = np.arange(n)
                ch = base + idx // C
                lane = idx % C
                iv = (vals - off).astype(np.int16)
                si16[lane % 16, ch * 8 + lane // 16] = iv
                db_[ch, lane] = dvals
        full = np.zeros((128, TC * 8), np.int16)
        for rr in range(8):
            full[rr * 16: (rr + 1) * 16] = si16
        per_core.append({
            "src16": full,                                       # [128, TC*8]
            "dst_col": np.ascontiguousarray(db_.T.astype(BF16_NP)),
            "dst_row": np.ascontiguousarray(
                db_.reshape(1, -1).astype(BF16_NP)),
        })
    return sched_lo, sched_hi, per_core


def fold_weights(wa, ba_, g, be, rm, rv, wb, bb, bn_eps=1e-5):
    wa = wa.astype(np.float64)
    A_i, A_j = wa[:, :C], wa[:, C:]
    s = g.astype(np.float64) / np.sqrt(rv.astype(np.float64) + bn_eps)
    wb64 = wb.astype(np.float64)
    wu_t = (A_i - A_j).T
    wv_t = A_j.T
    wb2 = s[:, None] * wb64.T
    c0 = bb.astype(np.float64) + (be.astype(np.float64) - rm.astype(np.float64) * s) @ wb64.T
    return (wu_t.astype(BF16_NP), wv_t.astype(BF16_NP),
            ba_.astype(BF16_NP).reshape(1, C),
            wb2.astype(BF16_NP), c0.astype(BF16_NP).reshape(1, C))


def make_consts():
    ic = np.arange(C, dtype=np.float32).reshape(C, 1)
    ir = np.tile(np.arange(C, dtype=np.float64), (C, GRP, 1)).astype(BF16_NP)
    ones = np.ones((C, 1), dtype=np.float32)
    return ic, ir, ones


# ======================================================================
# Full-problem kernel: 2-layer EdgeConv encoder, N=50000, E=600000, C=128
# ======================================================================

import os

N_NODES = 50000
N_EDGES = 600000
CORES = 8
BPC = 49                  # blocks per core
NBT = CORES * BPC         # 392 blocks total
NP = NBT * C              # padded node count 50176
BN_EPS = 1e-5

LAST = {}                 # timing/info stash for test harness


def _prep_all(x, edge_index):
    src = np.asarray(edge_index[0], np.int64).astype(np.int32)
    dst = np.asarray(edge_index[1], np.int64).astype(np.int32)
    sched_lo, sched_hi, per_core = prep_edges(src, dst, CORES, BPC, NBT)
    deg_full = np.bincount(dst, minlength=NP).astype(np.float64)
    x_pad = np.zeros((NP, C), np.float32)
    x_pad[:N_NODES] = x
    xt = np.ascontiguousarray(x_pad.T).astype(BF16_NP)
    return sched_lo, sched_hi, per_core, deg_full, xt


def _layer_inputs(xt_bf16, per_core, deg_full, wset):
    wu_t, wv_t, ba_f, wb2, c0 = wset
    ic, ir, ones = make_consts()
    onesr = np.ones((1, C), dtype=BF16_NP)
    onesrf = np.ones((1, C), np.float32)
    in_maps = []
    for k in range(CORES):
        npc = BPC * C
        in_maps.append({
            "xt": xt_bf16,
            "xt_own": np.ascontiguousarray(xt_bf16[:, k * npc: (k + 1) * npc]),
            "wv_t": wv_t, "wu_t": wu_t, "ba": ba_f, "wb2": wb2, "c0": c0,
            "iota_col": ic, "iota_row": ir,
            "deg": np.ascontiguousarray(
                deg_full[k * npc: (k + 1) * npc].reshape(1, npc).astype(BF16_NP)),
            "ones_col": ones, "ones_row": onesr, "ones_row_f32": onesrf,
            "src16": per_core[k]["src16"],
            "dst_row": per_core[k]["dst_row"],
            "dst_col": per_core[k]["dst_col"],
        })
    return in_maps


_NTFF_HOOK = None


def _get_ntff_hook():
    """Recreate the axon NTFF profile hook (antenv.axon_hooks is absent
    in this image; trn_boot has the ctypes implementation)."""
    global _NTFF_HOOK
    if _NTFF_HOOK is None:
        sys.path.insert(0, "/root/.axon_site")
        from trn_agent_boot.trn_boot import _ntff_profile_via_ctypes
        _NTFF_HOOK = _ntff_profile_via_ctypes("/opt/axon/libaxon_pjrt.so")
    return _NTFF_HOOK


def _run(nc, in_maps):
    import tempfile
    from concourse import bass2jax
    trace = bool(int(os.environ.get("EDGECONV_TRACE", "0")))
    hook = _get_ntff_hook() if trace else None
    if hook is None:
        results = bass2jax.run_bass_via_pjrt(nc, in_maps, n_cores=CORES)
        LAST.setdefault("exec_ns", []).append(None)
        return results
    neff_dir = tempfile.mkdtemp(prefix="edgeconv_ntff_")
    with hook(neff_dir, [0]):
        results = bass2jax.run_bass_via_pjrt(nc, in_maps, n_cores=CORES)
    exec_ns = None
    try:
        import glob as _glob
        import gauge.profiler
        from concourse._compat import FishPath
        if _glob.glob(os.path.join(neff_dir, "*_body*.ntff")):
            profile = gauge.profiler.Profile(
                profile_path=FishPath(neff_dir), kernel_dev_mode=True,
                profile_on_exit=False, bass_kernel=nc.m,
                offline_processing=True, fname="*_body*")
            pr = profile.to_perfetto(model_index=(0,))
            if pr:
                exec_ns = pr[0].exec_time_ns
                LAST.setdefault("trace_paths", []).append(pr[0].trace_path)
    except Exception as e:  # profiling must never break the kernel
        LAST.setdefault("trace_errors", []).append(repr(e))
    LAST.setdefault("neff_dirs", []).append(neff_dir)
    LAST.setdefault("exec_ns", []).append(exec_ns)
    return results


def kernel(**inputs):
    x = np.asarray(inputs["x"], np.float32)
    edge_index = np.asarray(inputs["edge_index"])
    sched_lo, sched_hi, per_core, deg_full, xt = _prep_all(x, edge_index)

    w1 = fold_weights(np.asarray(inputs["w1a"]), np.asarray(inputs["b1a"]),
                      np.asarray(inputs["g1"]), np.asarray(inputs["be1"]),
                      np.asarray(inputs["rm1"]), np.asarray(inputs["rv1"]),
                      np.asarray(inputs["w1b"]), np.asarray(inputs["b1b"]),
                      BN_EPS)
    w2 = fold_weights(np.asarray(inputs["w2a"]), np.asarray(inputs["b2a"]),
                      np.asarray(inputs["g2"]), np.asarray(inputs["be2"]),
                      np.asarray(inputs["rm2"]), np.asarray(inputs["rv2"]),
                      np.asarray(inputs["w2b"]), np.asarray(inputs["b2b"]),
                      BN_EPS)

    nc1 = build_layer(NBT, BPC, sched_lo, sched_hi, apply_norm=True)
    r1 = _run(nc1, _layer_inputs(xt, per_core, deg_full, w1))
    hT = np.concatenate([np.asarray(r["out_t"]) for r in r1], axis=1)

    nc2 = build_layer(NBT, BPC, sched_lo, sched_hi, apply_norm=False)
    r2 = _run(nc2, _layer_inputs(np.ascontiguousarray(hT), per_core,
                                 deg_full, w2))
    outT = np.concatenate([np.asarray(r["out_t"], np.float32) for r in r2],
                          axis=1)

    return np.ascontiguousarray(outT.T[:N_NODES]).astype(np.float32)
